# revision 4
# baseline (speedup 1.0000x reference)
"""FP8Linear Trainium2 kernel.

Computes out = x @ (dequant(weight_fp8) * scale_w)^T + bias for
x: (4, 8192, 2048) bf16, weight_fp8: (2048, 2048) fp8_e4m3fn, scale_w: scalar f32,
bias: (2048,) bf16  ->  out: (4, 8192, 2048) bf16.

Strategy: data-parallel over rows. x flattens to (32768, 2048); each of the 8
NeuronCores gets a contiguous 4096-row slice, the (small, 4MB) weight is
replicated, and each core computes its slice of the output independently — no
collectives. Per core this is compute-bound: ~34 GFLOP of bf16 matmul (~437us
at peak) vs ~38MB of DMA traffic.

fp8 handling: the e4m3fn bytes are shipped as uint8 (the OCP format is NOT
TRN's FP8_EXP4 — values in (240, 448] would decode as NaN/Inf on TRN), and
dequantized on-device. The bf16 bit pattern of 2^-120 * fp8_value(b) equals
    16*b + 30720*(b >= 128)
(sign moves from bit 7 to bit 15; exponent field lands at the bf16 exponent
offset by -120; exact incl. subnormals), computed with three DVE ops in its
fp32 ALU domain, then one ACT copy with scale = 2^120 * bf16(scale_w)
produces weights bit-identical to the reference's (fp8 -> bf16)*bf16(scale_w).

Per 128-row output tile the PSUM accumulator is drained by the ACT engine as
a plain f32->bf16 copy (freeing PSUM banks without waiting on the DVE, which
is busy decoding during the prologue), and the bias is added afterwards on
the DVE in bf16 — the same round-then-add order as the reference.
"""

import sys

sys.path.insert(0, "/opt/trn_rl_repo")

import numpy as np
import ml_dtypes

import concourse.bass as bass
import concourse.mybir as mybir
import concourse.tile as tile
from concourse import bacc
from concourse.bass_utils import run_bass_kernel_spmd

P = 128
N_CORES = 8


def _ldw_key(ld):
    w = ld.ins[0]
    bap = getattr(w, "bass_ap", None)
    if bap is not None:
        ident = (bap.tensor.name, str(bap.offset), str(bap.ap), str(bap.dtype))
    else:
        ident = (w.memref, str(w.offset), str(w.ap), str(w.dtype))
    return ident + (
        str(ld.perf_mode), str(ld.is_transpose), str(ld.tile_position),
    )


def _strip_redundant_ldweights(ordered):
    """Drop InstLdweights whose weights AP matches the previous PE weight
    load with only matmuls in between (the PE array still holds them).
    Dropped-LDW dependencies are merged into the following matmult and
    dangling name references remapped."""
    total = dropped = 0
    for bb, insts in ordered.items():
        keep = []
        last_key = None
        pending = []  # dropped LDWs awaiting the next kept PE inst
        rename = {}
        for ins in insts:
            if isinstance(ins, mybir.InstLdweights):
                total += 1
                key = _ldw_key(ins)
                if key == last_key:
                    dropped += 1
                    pending.append(ins)
                    continue
                last_key = key
            elif isinstance(ins, mybir.InstMatmult):
                if pending:
                    for ld in pending:
                        ins.merge_dependencies_from(ld)
                        rename[ld.name] = ins.name
                    pending = []
            elif getattr(ins, "engine", None) == mybir.EngineType.PE:
                last_key = None  # unknown PE inst may clobber array state
            keep.append(ins)
        assert not pending, "dropped LDW with no following matmult"
        if rename:
            for ins in keep:
                ins.remap_dependency_names(rename)
        ordered[bb] = keep
    return total, dropped

# Full problem dims (hardcoded per the contract).
B, S, IN, OUT = 4, 8192, 2048, 2048
M_TOTAL = B * S
M_CORE = M_TOTAL // N_CORES


def emit_fp8linear(tc, out, x, w_t, scale, bias_row, M, IN_, OUT_, MB, NO,
                   opts=None):
    """Emit the per-core program.

    out:   [M, OUT_]  bf16 DRAM (ExternalOutput)
    x:     [M, IN_]   bf16 DRAM
    w_t:   [IN_, OUT_] uint8 DRAM — fp8e4m3fn bytes of W^T (d-major)
    scale: [1, 1]     f32 DRAM
    bias_row: [1, OUT_] bf16 DRAM
    MB: m-block rows (DMA-transpose granularity), NO: matmul moving free dim.
    """
    opts = opts or {}
    decode_gpsimd_every = opts.get("decode_gpsimd_every", 0)  # 0=DVE only
    skip_decode = opts.get("skip_decode", False)  # w_t is pre-decoded bf16
    xt_bufs = opts.get("xt_bufs", None)
    ramp = opts.get("ramp", ())  # leading m-block sizes, e.g. (128, 128, 256)
    split_queues = opts.get("split_queues", True)
    decode_chunks = opts.get("decode_chunks", 1)
    swar = opts.get("swar", False)
    arith_decode = opts.get("arith_decode", False)
    use_stt = opts.get("use_stt", False)
    defer_bias = opts.get("defer_bias", False)
    interleave = opts.get("interleave", False)
    og_units = opts.get("og_units", 4)  # og groups per psum unit (4 = whole m_sub)
    nc = tc.nc
    DT = IN_ // P  # contraction (d) tiles
    OG = OUT_ // NO  # output column groups
    MS = MB // P  # 128-row subtiles per m block
    # m-block row schedule: optional small leading blocks so the PE can start
    # before the full first 512-row transpose lands, then MB-row blocks.
    blocks = list(ramp)
    assert sum(blocks) % MB == 0 or not blocks
    blocks += [MB] * ((M - sum(blocks)) // MB)
    assert sum(blocks) == M
    # second HWDGE queue (ACT sequencer) for weight/bias/scale loads + stores
    dma_w = nc.scalar if split_queues else nc.sync
    dma_x = nc.sync
    dt_bf16 = mybir.dt.bfloat16
    dt_u8 = mybir.dt.uint8
    dt_u16 = mybir.dt.uint16
    dt_f32 = mybir.dt.float32
    Alu = mybir.AluOpType

    with (
        tc.tile_pool(name="const", bufs=1) as const,
        tc.tile_pool(name="wpool", bufs=1) as wpool,
        tc.tile_pool(name="wstage", bufs=opts.get("wstage_bufs", 2)) as wstage,
        tc.tile_pool(name="xT", bufs=xt_bufs or 2 * DT) as xp,
        tc.tile_pool(name="psum", bufs=8, space="PSUM") as pp,
        tc.tile_pool(name="obuf", bufs=opts.get("obuf_bufs", 4)) as op,
    ):
        # ---- constants: scale first — its tiny DMA + two DVE ops gate every
        # wT decode, so it must not queue behind the bias broadcast spray ----
        s_raw = const.tile([P, 1], dt_f32)
        dma_w.dma_start(s_raw[:], scale.to_broadcast((P, 1)))
        # round scale to bf16 (reference multiplies by bf16(scale_w)), then
        # fold in the 2^120 exponent-offset of the integer decode below.
        s_bf = const.tile([P, 1], dt_bf16)
        nc.vector.tensor_copy(s_bf[:], s_raw[:])
        s_eff = const.tile([P, 1], dt_f32)
        nc.vector.tensor_scalar(
            out=s_eff[:], in0=s_bf[:], scalar1=float(2.0**120), scalar2=None,
            op0=Alu.mult,
        )

        # head-start: the first few weight-byte loads go ahead of the block-0
        # transposes so the decode pipeline has work while transposes stream
        w_head = opts.get("w_head", 0)
        wu8_pre = {}
        for dt in range(w_head):
            wu8p = wstage.tile(
                [P, OUT_], dt_u8, tag="wu8", name=f"wu8_{dt}",
                bufs=opts.get("wu8_bufs"),
            )
            dma_w.dma_start(wu8p[:], w_t[dt * P:(dt + 1) * P, :])
            wu8_pre[dt] = wu8p

        # ---- block-0 x transposes + weight decode, interleaved on the DMA
        # queue so the PE's lhsT path and the decode pipeline fill together ----
        xts0 = []

        def emit_xt0(dt):
            xt = xp.tile([P, blocks[0]], dt_bf16, tag="xT", name=f"xT0_{dt}")
            dma_x.dma_start(
                xt[:], x[0:blocks[0], dt * P:(dt + 1) * P], transpose=True
            )
            xts0.append(xt)

        if not interleave:
            for dt in range(DT):
                emit_xt0(dt)

        # remaining weight-byte loads issue right after the block-0 transposes
        # (ahead of later blocks' transposes and the bias spray in the queue)
        if opts.get("w_rest_early"):
            for dt in range(w_head, DT):
                wu8p = wstage.tile(
                    [P, OUT_], dt_u8, tag="wu8", name=f"wu8_{dt}",
                    bufs=opts.get("wu8_bufs"),
                )
                dma_w.dma_start(wu8p[:], w_t[dt * P:(dt + 1) * P, :])
                wu8_pre[dt] = wu8p

        # batched weight-byte loads: one 3D-AP DMA per chunk of d-tiles —
        # the prologue is HWDGE issue-count bound (~0.6us per dma_start)
        w_chunks = opts.get("w_chunks", 0)
        wu8_chunks = []
        if w_chunks:
            G = DT // w_chunks
            for c in range(w_chunks):
                wc = wstage.tile(
                    [P, G, OUT_], dt_u8, tag="wu8c", name=f"wu8c_{c}",
                    bufs=w_chunks,
                )
                dma_w.dma_start(
                    wc[:],
                    w_t[c * G * P:(c + 1) * G * P, :].rearrange(
                        "(g p) c -> p g c", p=P
                    ),
                )
                wu8_chunks.append(wc)

        # wT[dt] = bf16 dequant of W^T[dt*P:(dt+1)*P, :]
        wts = []
        for dt in range(DT):
            if interleave:
                emit_xt0(dt)
            wt = wpool.tile([P, OUT_], dt_bf16, tag=f"wT{dt}", name=f"wT{dt}")
            if skip_decode:
                dma_w.dma_start(wt[:], w_t[dt * P:(dt + 1) * P, :])
                wts.append(wt)
                continue
            if dt in wu8_pre:
                wu8 = wu8_pre[dt]
            elif w_chunks:
                G = DT // w_chunks
                wu8 = wu8_chunks[dt // G][:, dt % G, :]
            else:
                wu8 = wstage.tile(
                    [P, OUT_], dt_u8, tag="wu8", name=f"wu8_{dt}",
                    bufs=opts.get("wu8_bufs"),
                )
                # SWDGE (gpsimd) rides separate descriptor rings, so weight
                # bytes stream concurrently with the x transposes on HWDGE
                w_eng = nc.gpsimd if opts.get("w_swdge") else dma_w
                w_eng.dma_start(wu8[:], w_t[dt * P:(dt + 1) * P, :])
            eng = (
                nc.gpsimd
                if decode_gpsimd_every and dt % decode_gpsimd_every == 0
                else nc.vector
            )
            if arith_decode and dt < opts.get("head_fine", 0):
                # first tiles: decode per og-slice to cut chain latency so the
                # PE's first matmuls start as early as possible
                t1 = wstage.tile([P, OUT_], dt_u16, tag="t1", name=f"t1_{dt}",
                                 bufs=opts.get("t1_bufs"))
                tb = wstage.tile([P, OUT_], dt_u16, tag="tb", name=f"tb_{dt}",
                                 bufs=opts.get("tb_bufs"))
                for ci in range(OG):
                    cs = slice(ci * NO, (ci + 1) * NO)
                    eng.tensor_scalar(
                        out=t1[:, cs], in0=wu8[:, cs], scalar1=128,
                        scalar2=30720, op0=Alu.is_ge, op1=Alu.mult,
                    )
                    eng.tensor_scalar(
                        out=tb[:, cs], in0=wu8[:, cs], scalar1=16,
                        scalar2=None, op0=Alu.mult,
                    )
                    eng.tensor_tensor(tb[:, cs], tb[:, cs], t1[:, cs], Alu.add)
                    nc.scalar.activation(
                        wt[:, cs], tb[:, cs].bitcast(dt_bf16),
                        mybir.ActivationFunctionType.Copy, scale=s_eff[:],
                    )
                wts.append(wt)
                continue
            if arith_decode:
                # bf16 bit pattern of 2^-120*fp8_value(b) == 16*b + 30720*(b>=128)
                # (sign moves from bit7 to bit15: -128*16 + 32768 = +30720).
                # Two DVE ops in the fp32 ALU domain, exact for these integers.
                t1 = wstage.tile(
                    [P, OUT_], dt_u16, tag="t1", name=f"t1_{dt}",
                    bufs=opts.get("t1_bufs"),
                )
                eng.tensor_scalar(
                    out=t1[:], in0=wu8[:], scalar1=128, scalar2=30720,
                    op0=Alu.is_ge, op1=Alu.mult,
                )
                tb = wstage.tile(
                    [P, OUT_], dt_u16, tag="tb", name=f"tb_{dt}",
                    bufs=opts.get("tb_bufs"),
                )
                if use_stt:
                    eng.scalar_tensor_tensor(
                        out=tb[:], in0=wu8[:], scalar=16.0, in1=t1[:],
                        op0=Alu.mult, op1=Alu.add,
                    )
                elif opts.get("act_mul16") and dt % 2 == 1 and (
                    dt >= opts.get("act_skip_head", 0)
                ):
                    # balance the decode pipeline: ACT computes 16*b for
                    # alternating tiles so DVE only does 2 of the 3 ops
                    nc.scalar.activation(
                        tb[:], wu8[:], mybir.ActivationFunctionType.Copy,
                        scale=16.0,
                    )
                    eng.tensor_tensor(tb[:], tb[:], t1[:], Alu.add)
                else:
                    eng.tensor_scalar(
                        out=tb[:], in0=wu8[:], scalar1=16, scalar2=None,
                        op0=Alu.mult,
                    )
                    eng.tensor_tensor(tb[:], tb[:], t1[:], Alu.add)
                nc.scalar.activation(
                    wt[:], tb[:].bitcast(dt_bf16),
                    mybir.ActivationFunctionType.Copy, scale=s_eff[:],
                )
                wts.append(wt)
                continue
            if swar:
                # decode byte-pairs in u16 lanes: even/odd bytes split into
                # strided halves of the bit-pattern tile (5 half-width ops
                # instead of 4 full-width + widening copy)
                pair = wu8[:].bitcast(dt_u16)  # [P, OUT_/2]
                tb = wstage.tile([P, OUT_], dt_u16, tag="tb", name=f"tb_{dt}")
                tbv = tb[:].rearrange("p (o two) -> p two o", two=2)
                m0 = wstage.tile([P, OUT_ // 2], dt_u16, tag="m0", name=f"m0_{dt}")
                s0 = wstage.tile([P, OUT_ // 2], dt_u16, tag="s0", name=f"s0_{dt}")
                m1 = wstage.tile([P, OUT_ // 2], dt_u16, tag="m1", name=f"m1_{dt}")
                eng.tensor_scalar(
                    out=m0[:], in0=pair, scalar1=4, scalar2=0x07F0,
                    op0=Alu.logical_shift_left, op1=Alu.bitwise_and,
                )
                eng.tensor_scalar(
                    out=s0[:], in0=pair, scalar1=0x80, scalar2=8,
                    op0=Alu.bitwise_and, op1=Alu.logical_shift_left,
                )
                eng.tensor_tensor(tbv[:, 0, :], m0[:], s0[:], Alu.bitwise_or)
                eng.tensor_scalar(
                    out=m1[:], in0=pair, scalar1=4, scalar2=0x07F0,
                    op0=Alu.logical_shift_right, op1=Alu.bitwise_and,
                )
                s1 = wstage.tile([P, OUT_ // 2], dt_u16, tag="s1", name=f"s1_{dt}")
                eng.tensor_scalar(
                    out=s1[:], in0=pair, scalar1=0x8000, scalar2=None,
                    op0=Alu.bitwise_and,
                )
                eng.tensor_tensor(tbv[:, 1, :], m1[:], s1[:], Alu.bitwise_or)
                nc.scalar.activation(
                    wt[:], tb[:].bitcast(dt_bf16),
                    mybir.ActivationFunctionType.Copy, scale=s_eff[:],
                )
                wts.append(wt)
                continue
            # decode in column chunks so the PE can start each o-group's
            # matmuls before the whole 2048-wide row block is decoded
            CW = OUT_ // decode_chunks
            wu16 = wstage.tile([P, OUT_], dt_u16, tag="wu16", name=f"wu16_{dt}")
            t1 = wstage.tile([P, OUT_], dt_u16, tag="t1", name=f"t1_{dt}")
            t2 = wstage.tile([P, OUT_], dt_u16, tag="t2", name=f"t2_{dt}")
            for ci in range(decode_chunks):
                cs = slice(ci * CW, (ci + 1) * CW)
                eng.tensor_copy(wu16[:, cs], wu8[:, cs])  # zero-extend u8->u16
                eng.tensor_scalar(
                    out=t1[:, cs], in0=wu16[:, cs], scalar1=4, scalar2=0x07F0,
                    op0=Alu.logical_shift_left, op1=Alu.bitwise_and,
                )
                eng.tensor_scalar(
                    out=t2[:, cs], in0=wu16[:, cs], scalar1=8, scalar2=0x8000,
                    op0=Alu.logical_shift_left, op1=Alu.bitwise_and,
                )
                eng.tensor_tensor(t1[:, cs], t1[:, cs], t2[:, cs], Alu.bitwise_or)
                nc.scalar.activation(
                    wt[:, cs], t1[:, cs].bitcast(dt_bf16),
                    mybir.ActivationFunctionType.Copy, scale=s_eff[:],
                )
            wts.append(wt)

        # bias is first needed by the earliest PSUM drain, well after decode
        bias_t = const.tile([P, OUT_], dt_bf16)
        dma_w.dma_start(bias_t[:], bias_row.to_broadcast((P, OUT_)))

        # ---- main loop ----
        brow = 0
        for bi, rows_b in enumerate(blocks):
            if bi == 0:
                xts = xts0
            else:
                xts = []
                for dt in range(DT):
                    xt = xp.tile(
                        [P, rows_b], dt_bf16, tag="xT", name=f"xT{bi}_{dt}"
                    )
                    dma_x.dma_start(
                        xt[:], x[brow:brow + rows_b, dt * P:(dt + 1) * P],
                        transpose=True,
                    )
                    xts.append(xt)
            for ms in range(rows_b // P):
                OH = og_units  # psum-group granularity (og groups per unit)
                rot = opts.get("dt_rotate", 0)
                psums = []
                for u in range(OG // OH):
                    dt0 = (u * rot) % DT if rot else 0
                    for j in range(DT):
                        dt = (dt0 + j) % DT
                        lhsT = xts[dt][:, ms * P:(ms + 1) * P]
                        for oi in range(OH):
                            og = u * OH + oi
                            if len(psums) <= og:
                                psums.append(
                                    pp.tile([P, NO], dt_f32, tag="ps",
                                            name=f"ps{og}")
                                )
                            nc.tensor.matmul(
                                psums[og][:], lhsT,
                                wts[dt][:, og * NO:(og + 1) * NO],
                                start=(j == 0), stop=(j == DT - 1),
                            )
                ot = op.tile([P, OUT_], dt_bf16, tag="ot")
                if defer_bias:
                    # drain PSUM on the (idle) ACT engine so banks free without
                    # waiting on DVE; the f32->bf16 round before the bias add
                    # matches the reference's einsum-then-add bf16 semantics.
                    for og in range(OG):
                        nc.scalar.activation(
                            ot[:, og * NO:(og + 1) * NO], psums[og][:],
                            mybir.ActivationFunctionType.Copy,
                        )
                    if opts.get("og_stores"):
                        row0 = brow + ms * P
                        for og in range(OG):
                            sl = slice(og * NO, (og + 1) * NO)
                            nc.vector.tensor_tensor(
                                ot[:, sl], ot[:, sl], bias_t[:, sl], Alu.add
                            )
                            dma_w.dma_start(out[row0:row0 + P, sl], ot[:, sl])
                        continue
                    nc.vector.tensor_tensor(ot[:], ot[:], bias_t[:], Alu.add)
                else:
                    for og in range(OG):
                        nc.vector.tensor_tensor(
                            ot[:, og * NO:(og + 1) * NO], psums[og][:],
                            bias_t[:, og * NO:(og + 1) * NO], Alu.add,
                        )
                row0 = brow + ms * P
                dma_w.dma_start(out[row0:row0 + P, :], ot[:])
            brow += rows_b


def emit_v2(tc, out, x_t, w_t, scale, bias_row, M, IN_, OUT_, MB, NO, opts=None):
    """v2: x pre-transposed on host (x_t [IN_, M] bf16, plain DMA loads) and
    weights fed to the PE directly as TRN fp8e4 (host passes halved-value
    e4m3fn bytes, always <= 240 so the OCP->TRN bit patterns agree; the
    missing 2x is folded into the drain scale). No on-device decode at all.
    """
    opts = opts or {}
    nc = tc.nc
    DT = IN_ // P
    OG = OUT_ // NO
    ramp = opts.get("ramp", ())
    blocks = list(ramp)
    blocks += [MB] * ((M - sum(blocks)) // MB)
    assert sum(blocks) == M
    og_units = opts.get("og_units", 1)
    w_chunks = opts.get("w_chunks", 0)  # if >0, batch w loads into 3D DMAs
    dt_bf16 = mybir.dt.bfloat16
    dt_f32 = mybir.dt.float32
    dt_fp8 = mybir.dt.float8e4
    Alu = mybir.AluOpType
    dma = nc.sync
    dma2 = nc.scalar if opts.get("split_queues") else dma

    with (
        tc.tile_pool(name="const", bufs=1) as const,
        tc.tile_pool(name="wpool", bufs=1) as wpool,
        tc.tile_pool(name="xT", bufs=opts.get("xt_bufs", 2 * DT)) as xp,
        tc.tile_pool(name="psum", bufs=8, space="PSUM") as pp,
        tc.tile_pool(name="obuf", bufs=opts.get("obuf_bufs", 4)) as op,
    ):
        # scale: s_eff = 2 * bf16(scale_w) as f32, ACT scale operand at drain
        s_raw = const.tile([P, 1], dt_f32)
        dma.dma_start(s_raw[:], scale.to_broadcast((P, 1)))
        s_bf = const.tile([P, 1], dt_bf16)
        nc.vector.tensor_copy(s_bf[:], s_raw[:])
        s_eff = const.tile([P, 1], dt_f32)
        nc.vector.tensor_scalar(
            out=s_eff[:], in0=s_bf[:], scalar1=2.0, scalar2=None, op0=Alu.mult,
        )

        # weight tiles: [128, OUT_] fp8, d-major — straight loads, no decode
        wts = []
        if w_chunks:
            G = DT // w_chunks
            for c in range(w_chunks):
                wc = wpool.tile([P, G, OUT_], dt_fp8, tag=f"w8c{c}", name=f"w8c{c}")
                dma2.dma_start(
                    wc[:],
                    w_t[c * G * P:(c + 1) * G * P, :].rearrange(
                        "(g p) c -> p g c", p=P
                    ),
                )
                for g in range(G):
                    wts.append(wc[:, g, :])
        else:
            for dt in range(DT):
                wt = wpool.tile([P, OUT_], dt_fp8, tag=f"w8_{dt}", name=f"w8_{dt}")
                dma2.dma_start(wt[:], w_t[dt * P:(dt + 1) * P, :])
                wts.append(wt[:])

        bias_t = const.tile([P, OUT_], dt_bf16)
        dma2.dma_start(bias_t[:], bias_row.to_broadcast((P, OUT_)))

        brow = 0
        for bi, rows_b in enumerate(blocks):
            xts = []
            for dt in range(DT):
                xt = xp.tile([P, rows_b], dt_bf16, tag="xT", name=f"xT{bi}_{dt}")
                dma.dma_start(
                    xt[:], x_t[dt * P:(dt + 1) * P, brow:brow + rows_b]
                )
                xts.append(xt)
            for ms in range(rows_b // P):
                OH = og_units
                psums = []
                for u in range(OG // OH):
                    for j in range(DT):
                        dt = j
                        lhsT = xts[dt][:, ms * P:(ms + 1) * P]
                        for oi in range(OH):
                            og = u * OH + oi
                            if len(psums) <= og:
                                psums.append(
                                    pp.tile([P, NO], dt_f32, tag="ps",
                                            name=f"ps{og}")
                                )
                            nc.tensor.matmul(
                                psums[og][:], lhsT,
                                wts[dt][:, og * NO:(og + 1) * NO],
                                start=(j == 0), stop=(j == DT - 1),
                            )
                ot = op.tile([P, OUT_], dt_bf16, tag="ot")
                row0 = brow + ms * P
                for og in range(OG):
                    sl = slice(og * NO, (og + 1) * NO)
                    nc.scalar.activation(
                        ot[:, sl], psums[og][:],
                        mybir.ActivationFunctionType.Copy, scale=s_eff[:],
                    )
                    nc.vector.tensor_tensor(
                        ot[:, sl], ot[:, sl], bias_t[:, sl], Alu.add
                    )
                    dma2.dma_start(out[row0:row0 + P, sl], ot[:, sl])
            brow += rows_b


def build_nc(M=M_CORE, IN_=IN, OUT_=OUT, MB=512, NO=512, opts=None):
    opts = opts or {}
    v2 = opts.get("v2", False)
    nc = bacc.Bacc(
        "TRN2", target_bir_lowering=False, debug=False, num_devices=N_CORES
    )
    if v2:
        x_d = nc.dram_tensor(
            "x_t", [IN_, M], mybir.dt.bfloat16, kind="ExternalInput"
        )
        w_d = nc.dram_tensor(
            "w_t", [IN_, OUT_], mybir.dt.float8e4, kind="ExternalInput"
        )
    else:
        x_d = nc.dram_tensor(
            "x", [M, IN_], mybir.dt.bfloat16, kind="ExternalInput"
        )
        w_dtype = (
            mybir.dt.bfloat16 if opts.get("skip_decode") else mybir.dt.uint8
        )
        w_d = nc.dram_tensor("w_t", [IN_, OUT_], w_dtype, kind="ExternalInput")
    s_d = nc.dram_tensor("scale", [1, 1], mybir.dt.float32, kind="ExternalInput")
    b_d = nc.dram_tensor("bias", [1, OUT_], mybir.dt.bfloat16, kind="ExternalInput")
    o_d = nc.dram_tensor("out", [M, OUT_], mybir.dt.bfloat16, kind="ExternalOutput")
    strip = opts.get("strip_ldw", False)
    orig_legalize = tile.tile_legalize
    if strip:
        def _patched(ordered, nc_):
            ordered = orig_legalize(ordered, nc_)
            total, dropped = _strip_redundant_ldweights(ordered)
            print(f"strip_ldw: dropped {dropped}/{total} ldweights")
            return ordered
        tile.tile_legalize = _patched
    try:
        with tile.TileContext(nc) as tc:
            emit = emit_v2 if v2 else emit_fp8linear
            emit(
                tc, o_d.ap(), x_d.ap(), w_d.ap(), s_d.ap(), b_d.ap(),
                M, IN_, OUT_, MB, NO, opts=opts,
            )
    finally:
        tile.tile_legalize = orig_legalize
    nc.compile()
    return nc


# value-halving LUT for e4m3fn bytes: TRN fp8e4 only reaches +-240, so the
# (240, 448] codes of OCP e4m3fn would decode as NaN/Inf on the PE. Halving
# every weight (exact for normals; RNE for the 3 subnormal codes) keeps all
# magnitudes <= 224 where the two formats agree bit-for-bit; the drain scale
# carries the missing 2x.
_HALF_LUT = None


def _half_fp8_bytes(b: np.ndarray) -> np.ndarray:
    global _HALF_LUT
    if _HALF_LUT is None:
        all_bytes = np.arange(256, dtype=np.uint8)
        vals = all_bytes.view(ml_dtypes.float8_e4m3fn).astype(np.float32)
        halves = (vals * 0.5).astype(ml_dtypes.float8_e4m3fn).view(np.uint8)
        _HALF_LUT = halves
    return _HALF_LUT[b]


_NC_CACHE = {}

# build options used by kernel(); test harnesses may override before first call
OPTS = {
    "split_queues": False, "arith_decode": True, "defer_bias": True,
    "og_units": 1, "wstage_bufs": 4, "wu8_bufs": 16, "t1_bufs": 2, "tb_bufs": 4,
    "og_stores": True, "ramp": (128, 384), "act_mul16": True, "w_head": 4,
}


def make_in_maps(x, weight_fp8, scale_w, bias, opts):
    """Host-side prep shared by kernel() and the bench harness."""
    x2 = np.asarray(x).reshape(M_TOTAL, IN)
    s = np.asarray(scale_w, dtype=np.float32).reshape(1, 1)
    b_row = np.ascontiguousarray(
        np.asarray(bias).reshape(1, OUT).astype(ml_dtypes.bfloat16, copy=False)
    )
    w_u8 = np.asarray(weight_fp8).view(np.uint8)
    if opts.get("v2"):
        # d-major halved weight bytes, valid TRN fp8e4
        w_t = np.ascontiguousarray(_half_fp8_bytes(w_u8).T).view(
            mybir.dt.np(mybir.dt.float8e4)
        )
        x_t = np.ascontiguousarray(
            x2.astype(ml_dtypes.bfloat16, copy=False).T
        )
        return [
            {
                "x_t": x_t[:, c * M_CORE:(c + 1) * M_CORE],
                "w_t": w_t,
                "scale": s,
                "bias": b_row,
            }
            for c in range(N_CORES)
        ]
    w_t_u8 = np.ascontiguousarray(w_u8.T)
    x2 = np.ascontiguousarray(x2.astype(ml_dtypes.bfloat16, copy=False))
    return [
        {
            "x": x2[c * M_CORE:(c + 1) * M_CORE],
            "w_t": w_t_u8,
            "scale": s,
            "bias": b_row,
        }
        for c in range(N_CORES)
    ]


def kernel(x, weight_fp8, scale_w, bias):
    x = np.asarray(x)
    weight_fp8 = np.asarray(weight_fp8)
    bias = np.asarray(bias)
    assert x.shape == (B, S, IN) and weight_fp8.shape == (OUT, IN)

    key = str(sorted(OPTS.items()))
    if key not in _NC_CACHE:
        _NC_CACHE[key] = build_nc(opts=dict(OPTS))
    nc = _NC_CACHE[key]

    in_maps = make_in_maps(x, weight_fp8, scale_w, bias, OPTS)
    res = run_bass_kernel_spmd(nc, in_maps, list(range(N_CORES)))
    shards = [res.results[c]["out"] for c in range(N_CORES)]
    out = np.concatenate(shards, axis=0).reshape(B, S, OUT)
    return out.astype(ml_dtypes.bfloat16, copy=False)

